# revision 12
# baseline (speedup 1.0000x reference)
"""CRF NLL kernel for Trainium2 (8 NeuronCores, batch-parallel).

Math: the CRF forward recursion
    part_t[j] = logsumexp_i(part_{t-1}[i] + trans[i,j]) + feat[t,j]
is run in the exponential domain:
    p_t[j,b] = (sum_i p_{t-1}[i,b] * E[i,j]) * F_t[j,b]
with E = exp(trans) and F_t = exp(feat_t - lognorm_t) the *normalized*
emission weights (per-(t,b) log-normalizers are folded back in on the
host).

The serial scan over seq_len is broken with a Perron-Frobenius stripe
decomposition: products of strictly positive matrices contract the
projective (Hilbert) metric geometrically — for E = exp(0.1*randn) a
single step washes out the initial direction to below bf16 rounding
noise. Each sequence is split into R overlapping time-stripes; stripe
r>=1 starts from a uniform state W steps before its real region, so
its trajectory equals the true one up to one unknown per-stripe
scalar. The host recovers the scalars by chaining L1-norm ratios at
the overlap times (within-stripe ratios are exact: the scalar
cancels); stripe 0 starts from the exact p_0, anchoring the absolute
scale.

Device work: K = L+W steps of one [64x64] @ [64 x 8*R] bf16 matmul
(E stationary) plus one DVE multiply PSUM*F -> bf16 SBUF, instead of
s_eff serial steps of an 8-wide matmul. Chain length 9 instead of 252;
per-step instruction overheads (PE fixed SBUF access ~173ns, DVE PSUM
access ~125ns, semaphores) dominate, so the 32x wider ops are nearly
free. The host pre-packs the per-(stripe, seq) emission schedule so
the device kernel is a plain dense scan. Inputs arrive in two batched
DMAs; trajectory blocks are stored in three grouped DMAs streamed
behind the scan.
"""

import sys

sys.path.insert(0, "/opt/trn_rl_repo")

import numpy as np

B, S, TAG = 64, 256, 64
START, END = TAG - 2, TAG - 1
NCORES = 8
BLOC = B // NCORES  # 8 sequences per core

R = 128  # stripes per sequence
W = 1    # warmup steps per stripe

_compiled = {}


def _plan(s_eff):
    """Stripe geometry: L real steps per stripe, K=L+W chain steps."""
    L = max(1, -(-(s_eff - W) // R))  # ceil((s_eff-W)/R)
    K = L + W
    return L, K


def _build_nc(K):
    import concourse.bass as bass
    import concourse.bacc as bacc
    import concourse.mybir as mybir
    from concourse import tile

    f32 = mybir.dt.float32
    bf16 = mybir.dt.bfloat16
    nc = bacc.Bacc(
        "TRN2", target_bir_lowering=False, debug=False, num_devices=NCORES
    )

    CW = R * BLOC                   # columns per step-block
    NIN = TAG + (K + 1) * CW        # [E | init block | step blocks 1..K]
    NOUT = K * CW                   # states after steps 1..K
    ft_d = nc.dram_tensor("ft", [TAG, NIN], bf16, kind="ExternalInput")
    out_d = nc.dram_tensor("out", [TAG, NOUT], bf16, kind="ExternalOutput")

    def bcol(k):  # first ft column of step-block k
        return TAG + k * CW

    # input DMA batches (column ranges): E + the first two blocks arrive in
    # one transfer so a single DMA completion gates the first matmul; the
    # rest is split so later blocks' semaphores land before their TT needs
    # them (per-DMA latency is ~2.2us: DGE gen + engine delay + sem prop).
    in_batches = [(0, bcol(1)), (bcol(1), bcol(K)), (bcol(K), NIN)]
    in_batches = [(a, b) for a, b in in_batches if a < b]
    # output DMA groups (1-indexed step blocks), issued as the scan passes;
    # the last group is a single block (stored per half-chain) to shorten
    # the post-scan tail
    out_groups = [(1, K // 2), (K // 2, K), (K, K + 1)]
    out_groups = [(a, b) for a, b in out_groups if a < b]

    CH = CW // 2  # per-chain width: two interleaved chains overlap PE and DVE

    with tile.TileContext(nc) as tc:
        with (
            tc.tile_pool(name="pool", bufs=1) as pool,
            tc.tile_pool(name="psum", bufs=4, space=bass.MemorySpace.PSUM) as psum,
        ):
            ft_t = pool.tile([TAG, NIN], bf16)
            snap = pool.tile([TAG, NOUT], bf16)

            # DRAM loads land directly in ft_t; consumers wait on the DMA
            # queue semaphore (bacc hoists extra matmul waits onto the
            # LDWEIGHTS slot, and the scheduler inserts standalone waits
            # where an instruction needs more than one).
            for lo, hi in in_batches:
                nc.sync.dma_start(ft_t[:, lo:hi], ft_d[:, lo:hi])

            gi = 0
            for t in range(1, K + 1):
                for h in range(2):
                    ps = psum.tile([TAG, CH], f32)
                    o = h * CH
                    rhs = (
                        ft_t[:, bcol(0) + o : bcol(0) + o + CH]
                        if t == 1
                        else snap[:, (t - 2) * CW + o : (t - 2) * CW + o + CH]
                    )
                    nc.tensor.matmul(ps[:], ft_t[:, 0:TAG], rhs)
                    nc.vector.tensor_mul(
                        snap[:, (t - 1) * CW + o : (t - 1) * CW + o + CH],
                        ps[:],
                        ft_t[:, bcol(t) + o : bcol(t) + o + CH],
                    )
                    if t == K:
                        # last block: store each half as soon as its TT lands
                        nc.gpsimd.dma_start(
                            out_d[:, (t - 1) * CW + o : (t - 1) * CW + o + CH],
                            snap[:, (t - 1) * CW + o : (t - 1) * CW + o + CH],
                        )
                while gi < len(out_groups) and out_groups[gi][1] - 1 == t and t < K:
                    a, b2 = out_groups[gi]
                    nc.gpsimd.dma_start(
                        out_d[:, (a - 1) * CW : (b2 - 1) * CW],
                        snap[:, (a - 1) * CW : (b2 - 1) * CW],
                    )
                    gi += 1

    nc.compile()
    return nc


def _get_nc(K):
    if K not in _compiled:
        _compiled[K] = _build_nc(K)
    return _compiled[K]


def _run_device(in_maps, K, trace=False):
    from concourse.bass_utils import run_bass_kernel_spmd

    nc = _get_nc(K)
    return run_bass_kernel_spmd(nc, in_maps, list(range(NCORES)), trace=trace)


def _logsumexp(x, axis=-1):
    m = np.max(x, axis=axis, keepdims=True)
    return np.squeeze(m, axis) + np.log(np.sum(np.exp(x - m), axis=axis))


def prepare_inputs(feats, transitions, s_eff):
    """Host-side prep: normalized emissions packed in stripe order.

    Column layout within a block: col = r*BLOC + bl  (stripe-major).
    Stripe r's chain step k (1..K) applies the emission at absolute time
    t_abs = t0_r + k, clamped to s_eff-1, where t0_0 = 0 and
    t0_r = r*L - W.  Block 0 holds the init states.
    Returns (in_maps, lognorm, p0) — p0 in float64 for the host gather.
    """
    import ml_dtypes

    L, K = _plan(s_eff)
    CW = R * BLOC
    feats64 = feats.astype(np.float64)
    lognorm = _logsumexp(feats64, axis=2)  # (B,S)
    fnorm = np.exp(feats64 - lognorm[:, :, None])  # (B,S,T) float64
    tr = transitions.astype(np.float64)
    e_mat = np.ascontiguousarray(np.exp(tr).astype(np.float32))  # (T,T) rows=i
    es = np.exp(tr[START, :])  # (T,)
    p0 = fnorm[:, 0, :] * es[None, :]  # (B,T) exact init, float64

    t0s = np.array([0] + [r * L - W for r in range(1, R)])  # (R,)
    t_abs = np.clip(t0s[:, None] + np.arange(1, K + 1)[None, :], 0, s_eff - 1)

    bf = ml_dtypes.bfloat16
    in_maps = []
    for c in range(NCORES):
        sl = slice(c * BLOC, (c + 1) * BLOC)
        ftc = np.empty((TAG, TAG + (K + 1) * CW), dtype=bf)
        ftc[:, :TAG] = e_mat.astype(bf)
        blk0 = np.ones((R, BLOC, TAG), dtype=np.float64)
        blk0[0] = p0[sl]
        ftc[:, TAG : TAG + CW] = blk0.reshape(CW, TAG).T.astype(bf)
        sched = fnorm[sl][:, t_abs, :]        # (BLOC, R, K, TAG)
        sched = sched.transpose(3, 2, 1, 0)   # (TAG, K, R, BLOC)
        ftc[:, TAG + CW :] = sched.reshape(TAG, K * CW).astype(bf)
        in_maps.append({"ft": np.ascontiguousarray(ftc)})
    return in_maps, lognorm, p0


def finish(results, lognorm, p0, s_eff, feats, mask, tags, transitions):
    """Calibrate stripe scales, gather per-length states, compute NLL.

    Device out column for the state after chain step k (1..K) of
    (stripe r, lane bl): (k-1)*CW + r*BLOC + bl.
    """
    mask = np.asarray(mask).astype(bool)
    tags = np.asarray(tags).astype(np.int64)
    tr = np.asarray(transitions).astype(np.float64)
    lengths = mask.sum(axis=1).astype(np.int64)
    L, K = _plan(s_eff)
    CW = R * BLOC
    t0s = [0] + [r * L - W for r in range(1, R)]

    fwd = 0.0
    with np.errstate(divide="ignore"):
        for c in range(NCORES):
            out = np.asarray(results[c]["out"]).astype(np.float64)
            for bl in range(BLOC):
                b = c * BLOC + bl
                logscale = np.zeros(R)
                for r in range(1, R):
                    k_r = W                      # stripe r at time r*L
                    k_rm = K if r > 1 else L     # stripe r-1 at time r*L
                    num = out[:, (k_rm - 1) * CW + (r - 1) * BLOC + bl].sum()
                    den = out[:, (k_r - 1) * CW + r * BLOC + bl].sum()
                    logscale[r] = logscale[r - 1] + np.log(num) - np.log(den)
                tb = int(lengths[b]) - 1
                if tb == 0:
                    part = np.log(p0[b]) + lognorm[b, 0]
                else:
                    r = 0 if tb < K else min(tb // L, R - 1)
                    k = tb - t0s[r]              # chain step (1..K)
                    pv = out[:, (k - 1) * CW + r * BLOC + bl]
                    part = np.log(pv) + logscale[r] + lognorm[b, : tb + 1].sum()
                fwd += _logsumexp(part + tr[:, END])

    feats64 = np.asarray(feats).astype(np.float64)
    prev = np.concatenate(
        [np.full((B, 1), START, dtype=np.int64), tags[:, :-1]], axis=1
    )
    emit = np.take_along_axis(feats64, tags[:, :, None], axis=2)[:, :, 0]
    trans_sc = tr[prev, tags]
    tg = np.where(mask, emit + trans_sc, 0.0).sum()
    end_ids = tags[np.arange(B), lengths - 1]
    gold = tg + tr[end_ids, END].sum()

    return np.float32(fwd - gold)


def kernel(feats, mask, tags, transitions):
    feats = np.asarray(feats, dtype=np.float32)
    transitions = np.asarray(transitions, dtype=np.float32)
    s_eff = int(np.asarray(mask).astype(bool).sum(axis=1).max())
    _, K = _plan(s_eff)
    in_maps, lognorm, p0 = prepare_inputs(feats, transitions, s_eff)
    res = _run_device(in_maps, K).results
    return finish(res, lognorm, p0, s_eff, feats, mask, tags, transitions)


# revision 14
# speedup vs baseline: 1.0369x; 1.0369x over previous
"""CRF NLL kernel for Trainium2 (8 NeuronCores, batch-parallel).

Math: the CRF forward recursion
    part_t[j] = logsumexp_i(part_{t-1}[i] + trans[i,j]) + feat[t,j]
is run in the exponential domain:
    p_t[j,b] = (sum_i p_{t-1}[i,b] * E[i,j]) * F_t[j,b]
with E = exp(trans) and F_t = exp(feat_t - lognorm_t) the *normalized*
emission weights (per-(t,b) log-normalizers are folded back in on the
host).

The serial scan over seq_len is broken with a Perron-Frobenius stripe
decomposition: products of strictly positive matrices contract the
projective (Hilbert) metric geometrically — for E = exp(0.1*randn) a
single step washes out the initial direction to below bf16 rounding
noise. Each sequence is split into R overlapping time-stripes; stripe
r>=1 starts from a uniform state W steps before its real region, so
its trajectory equals the true one up to one unknown per-stripe
scalar. The host recovers the scalars by chaining L1-norm ratios at
the overlap times (within-stripe ratios are exact: the scalar
cancels); stripe 0 starts from the exact p_0, anchoring the absolute
scale.

Device work: K = L+W steps of one [64x64] @ [64 x 8*R] bf16 matmul
(E stationary) plus one DVE multiply PSUM*F -> bf16 SBUF, instead of
s_eff serial steps of an 8-wide matmul. Chain length 9 instead of 252;
per-step instruction overheads (PE fixed SBUF access ~173ns, DVE PSUM
access ~125ns, semaphores) dominate, so the 32x wider ops are nearly
free. The host pre-packs the per-(stripe, seq) emission schedule so
the device kernel is a plain dense scan. Inputs arrive in two batched
DMAs; trajectory blocks are stored in three grouped DMAs streamed
behind the scan.
"""

import sys

sys.path.insert(0, "/opt/trn_rl_repo")

import numpy as np

B, S, TAG = 64, 256, 64
START, END = TAG - 2, TAG - 1
NCORES = 8
BLOC = B // NCORES  # 8 sequences per core

R = 64  # stripes per sequence
W = 1   # warmup steps per stripe

_compiled = {}


def _plan(s_eff):
    """Stripe geometry: L real steps per stripe, K=L+W chain steps."""
    L = max(1, -(-(s_eff - W) // R))  # ceil((s_eff-W)/R)
    K = L + W
    return L, K


def _build_nc(K):
    import concourse.bass as bass
    import concourse.bacc as bacc
    import concourse.mybir as mybir
    from concourse import tile

    f32 = mybir.dt.float32
    bf16 = mybir.dt.bfloat16
    nc = bacc.Bacc(
        "TRN2", target_bir_lowering=False, debug=False, num_devices=NCORES
    )

    CW = R * BLOC                   # columns per step-block
    NIN = TAG + (K + 1) * CW        # [E | init block | step blocks 1..K]
    NOUT = K * CW                   # states after steps 1..K
    ft_d = nc.dram_tensor("ft", [TAG, NIN], bf16, kind="ExternalInput")
    out_d = nc.dram_tensor("out", [TAG, NOUT], bf16, kind="ExternalOutput")

    def bcol(k):  # first ft column of step-block k
        return TAG + k * CW

    # input DMA batches (column ranges): E + the first two blocks arrive in
    # one transfer so a single DMA completion gates the first matmul; the
    # rest is split so later blocks' semaphores land before their TT needs
    # them (per-DMA latency is ~2.2us: DGE gen + engine delay + sem prop).
    in_batches = [(0, bcol(1)), (bcol(1), bcol(2))] + [
        (bcol(k), bcol(min(k + 2, K + 1))) for k in range(2, K + 1, 2)
    ]
    in_batches = [(a, b) for a, b in in_batches if a < b]
    # output DMA groups (1-indexed step blocks), issued as the scan passes;
    # the last group is a single block (stored per half-chain) to shorten
    # the post-scan tail
    out_groups = [(1, K // 2), (K // 2, K), (K, K + 1)]
    out_groups = [(a, b) for a, b in out_groups if a < b]

    CH = CW // 2  # per-chain width: two interleaved chains overlap PE and DVE

    with tile.TileContext(nc) as tc:
        with (
            tc.tile_pool(name="pool", bufs=1) as pool,
            tc.tile_pool(name="psum", bufs=4, space=bass.MemorySpace.PSUM) as psum,
        ):
            ft_t = pool.tile([TAG, NIN], bf16)
            snap = pool.tile([TAG, NOUT], bf16)

            # DRAM loads land directly in ft_t; consumers wait on the DMA
            # queue semaphore (bacc hoists extra matmul waits onto the
            # LDWEIGHTS slot, and the scheduler inserts standalone waits
            # where an instruction needs more than one).
            for lo, hi in in_batches:
                nc.sync.dma_start(ft_t[:, lo:hi], ft_d[:, lo:hi])

            gi = 0
            for t in range(1, K + 1):
                for h in range(2):
                    ps = psum.tile([TAG, CH], f32)
                    o = h * CH
                    rhs = (
                        ft_t[:, bcol(0) + o : bcol(0) + o + CH]
                        if t == 1
                        else snap[:, (t - 2) * CW + o : (t - 2) * CW + o + CH]
                    )
                    nc.tensor.matmul(ps[:], ft_t[:, 0:TAG], rhs)
                    nc.vector.tensor_mul(
                        snap[:, (t - 1) * CW + o : (t - 1) * CW + o + CH],
                        ps[:],
                        ft_t[:, bcol(t) + o : bcol(t) + o + CH],
                    )
                    if t == K:
                        # last block: store each half as soon as its TT lands
                        nc.gpsimd.dma_start(
                            out_d[:, (t - 1) * CW + o : (t - 1) * CW + o + CH],
                            snap[:, (t - 1) * CW + o : (t - 1) * CW + o + CH],
                        )
                while gi < len(out_groups) and out_groups[gi][1] - 1 == t and t < K:
                    a, b2 = out_groups[gi]
                    nc.gpsimd.dma_start(
                        out_d[:, (a - 1) * CW : (b2 - 1) * CW],
                        snap[:, (a - 1) * CW : (b2 - 1) * CW],
                    )
                    gi += 1

    nc.compile()
    return nc


def _get_nc(K):
    if K not in _compiled:
        _compiled[K] = _build_nc(K)
    return _compiled[K]


def _run_device(in_maps, K, trace=False):
    from concourse.bass_utils import run_bass_kernel_spmd

    nc = _get_nc(K)
    return run_bass_kernel_spmd(nc, in_maps, list(range(NCORES)), trace=trace)


def _logsumexp(x, axis=-1):
    m = np.max(x, axis=axis, keepdims=True)
    return np.squeeze(m, axis) + np.log(np.sum(np.exp(x - m), axis=axis))


def prepare_inputs(feats, transitions, s_eff):
    """Host-side prep: normalized emissions packed in stripe order.

    Column layout within a block: col = r*BLOC + bl  (stripe-major).
    Stripe r's chain step k (1..K) applies the emission at absolute time
    t_abs = t0_r + k, clamped to s_eff-1, where t0_0 = 0 and
    t0_r = r*L - W.  Block 0 holds the init states.
    Returns (in_maps, lognorm, p0) — p0 in float64 for the host gather.
    """
    import ml_dtypes

    L, K = _plan(s_eff)
    CW = R * BLOC
    feats64 = feats.astype(np.float64)
    lognorm = _logsumexp(feats64, axis=2)  # (B,S)
    fnorm = np.exp(feats64 - lognorm[:, :, None])  # (B,S,T) float64
    tr = transitions.astype(np.float64)
    e_mat = np.ascontiguousarray(np.exp(tr).astype(np.float32))  # (T,T) rows=i
    es = np.exp(tr[START, :])  # (T,)
    p0 = fnorm[:, 0, :] * es[None, :]  # (B,T) exact init, float64

    t0s = np.array([0] + [r * L - W for r in range(1, R)])  # (R,)
    t_abs = np.clip(t0s[:, None] + np.arange(1, K + 1)[None, :], 0, s_eff - 1)

    bf = ml_dtypes.bfloat16
    in_maps = []
    for c in range(NCORES):
        sl = slice(c * BLOC, (c + 1) * BLOC)
        ftc = np.empty((TAG, TAG + (K + 1) * CW), dtype=bf)
        ftc[:, :TAG] = e_mat.astype(bf)
        blk0 = np.ones((R, BLOC, TAG), dtype=np.float64)
        blk0[0] = p0[sl]
        ftc[:, TAG : TAG + CW] = blk0.reshape(CW, TAG).T.astype(bf)
        sched = fnorm[sl][:, t_abs, :]        # (BLOC, R, K, TAG)
        sched = sched.transpose(3, 2, 1, 0)   # (TAG, K, R, BLOC)
        ftc[:, TAG + CW :] = sched.reshape(TAG, K * CW).astype(bf)
        in_maps.append({"ft": np.ascontiguousarray(ftc)})
    return in_maps, lognorm, p0


def finish(results, lognorm, p0, s_eff, feats, mask, tags, transitions):
    """Calibrate stripe scales, gather per-length states, compute NLL.

    Device out column for the state after chain step k (1..K) of
    (stripe r, lane bl): (k-1)*CW + r*BLOC + bl.
    """
    mask = np.asarray(mask).astype(bool)
    tags = np.asarray(tags).astype(np.int64)
    tr = np.asarray(transitions).astype(np.float64)
    lengths = mask.sum(axis=1).astype(np.int64)
    L, K = _plan(s_eff)
    CW = R * BLOC
    t0s = [0] + [r * L - W for r in range(1, R)]

    fwd = 0.0
    with np.errstate(divide="ignore"):
        for c in range(NCORES):
            out = np.asarray(results[c]["out"]).astype(np.float64)
            for bl in range(BLOC):
                b = c * BLOC + bl
                logscale = np.zeros(R)
                for r in range(1, R):
                    k_r = W                      # stripe r at time r*L
                    k_rm = K if r > 1 else L     # stripe r-1 at time r*L
                    num = out[:, (k_rm - 1) * CW + (r - 1) * BLOC + bl].sum()
                    den = out[:, (k_r - 1) * CW + r * BLOC + bl].sum()
                    logscale[r] = logscale[r - 1] + np.log(num) - np.log(den)
                tb = int(lengths[b]) - 1
                if tb == 0:
                    part = np.log(p0[b]) + lognorm[b, 0]
                else:
                    r = 0 if tb < K else min(tb // L, R - 1)
                    k = tb - t0s[r]              # chain step (1..K)
                    pv = out[:, (k - 1) * CW + r * BLOC + bl]
                    part = np.log(pv) + logscale[r] + lognorm[b, : tb + 1].sum()
                fwd += _logsumexp(part + tr[:, END])

    feats64 = np.asarray(feats).astype(np.float64)
    prev = np.concatenate(
        [np.full((B, 1), START, dtype=np.int64), tags[:, :-1]], axis=1
    )
    emit = np.take_along_axis(feats64, tags[:, :, None], axis=2)[:, :, 0]
    trans_sc = tr[prev, tags]
    tg = np.where(mask, emit + trans_sc, 0.0).sum()
    end_ids = tags[np.arange(B), lengths - 1]
    gold = tg + tr[end_ids, END].sum()

    return np.float32(fwd - gold)


def kernel(feats, mask, tags, transitions):
    feats = np.asarray(feats, dtype=np.float32)
    transitions = np.asarray(transitions, dtype=np.float32)
    s_eff = int(np.asarray(mask).astype(bool).sum(axis=1).max())
    _, K = _plan(s_eff)
    in_maps, lognorm, p0 = prepare_inputs(feats, transitions, s_eff)
    res = _run_device(in_maps, K).results
    return finish(res, lognorm, p0, s_eff, feats, mask, tags, transitions)


# revision 15
# speedup vs baseline: 1.0507x; 1.0133x over previous
"""CRF NLL kernel for Trainium2 (8 NeuronCores, batch-parallel).

Math: the CRF forward recursion
    part_t[j] = logsumexp_i(part_{t-1}[i] + trans[i,j]) + feat[t,j]
is run in the exponential domain:
    p_t[j,b] = (sum_i p_{t-1}[i,b] * E[i,j]) * F_t[j,b]
with E = exp(trans) and F_t = exp(feat_t - lognorm_t) the *normalized*
emission weights (per-(t,b) log-normalizers are folded back in on the
host).

The serial scan over seq_len is broken with a Perron-Frobenius stripe
decomposition: products of strictly positive matrices contract the
projective (Hilbert) metric geometrically — for E = exp(0.1*randn) a
single step washes out the initial direction to below bf16 rounding
noise. Each sequence is split into R overlapping time-stripes; stripe
r>=1 starts from a uniform state W steps before its real region, so
its trajectory equals the true one up to one unknown per-stripe
scalar. The host recovers the scalars by chaining L1-norm ratios at
the overlap times (within-stripe ratios are exact: the scalar
cancels); stripe 0 starts from the exact p_0, anchoring the absolute
scale.

Device work: K = L+W steps of one [64x64] @ [64 x 8*R] bf16 matmul
(E stationary) plus one DVE multiply PSUM*F -> bf16 SBUF, instead of
s_eff serial steps of an 8-wide matmul. Chain length 9 instead of 252;
per-step instruction overheads (PE fixed SBUF access ~173ns, DVE PSUM
access ~125ns, semaphores) dominate, so the 32x wider ops are nearly
free. The host pre-packs the per-(stripe, seq) emission schedule so
the device kernel is a plain dense scan. Inputs arrive in two batched
DMAs; trajectory blocks are stored in three grouped DMAs streamed
behind the scan.
"""

import sys

sys.path.insert(0, "/opt/trn_rl_repo")

import numpy as np

B, S, TAG = 64, 256, 64
START, END = TAG - 2, TAG - 1
NCORES = 8
BLOC = B // NCORES  # 8 sequences per core

R = 64  # stripes per sequence
W = 1   # warmup steps per stripe

_compiled = {}


def _plan(s_eff):
    """Stripe geometry: L real steps per stripe, K=L+W chain steps."""
    L = max(1, -(-(s_eff - W) // R))  # ceil((s_eff-W)/R)
    K = L + W
    return L, K


def _build_nc(K):
    import concourse.bass as bass
    import concourse.bacc as bacc
    import concourse.mybir as mybir
    from concourse import tile

    f32 = mybir.dt.float32
    bf16 = mybir.dt.bfloat16
    nc = bacc.Bacc(
        "TRN2", target_bir_lowering=False, debug=False, num_devices=NCORES
    )

    CW = R * BLOC                   # columns per step-block
    NIN = TAG + (K + 1) * CW        # [E | init block | step blocks 1..K]
    NOUT = K * CW                   # states after steps 1..K
    ft_d = nc.dram_tensor("ft", [TAG, NIN], bf16, kind="ExternalInput")
    out_d = nc.dram_tensor("out", [TAG, NOUT], bf16, kind="ExternalOutput")

    def bcol(k):  # first ft column of step-block k
        return TAG + k * CW

    # input DMA batches (column ranges): E + the first two blocks arrive in
    # one transfer so a single DMA completion gates the first matmul; the
    # rest is split so later blocks' semaphores land before their TT needs
    # them (per-DMA latency is ~2.2us: DGE gen + engine delay + sem prop).
    in_batches = [(0, bcol(1)), (bcol(1), bcol(2))] + [
        (bcol(k), bcol(min(k + 2, K + 1))) for k in range(2, K + 1, 2)
    ]
    in_batches = [(a, b) for a, b in in_batches if a < b]
    # output DMA groups (1-indexed step blocks), issued as the scan passes;
    # the last group is a single block (stored per half-chain) to shorten
    # the post-scan tail
    out_groups = [(1, K // 2), (K // 2, K), (K, K + 1)]
    out_groups = [(a, b) for a, b in out_groups if a < b]

    CH = CW // 2  # per-chain width: two interleaved chains overlap PE and DVE

    with tile.TileContext(nc) as tc:
        with (
            tc.tile_pool(name="pool", bufs=1) as pool,
            tc.tile_pool(name="psum", bufs=4, space=bass.MemorySpace.PSUM) as psum,
        ):
            ft_t = pool.tile([TAG, NIN], bf16)
            snap = pool.tile([TAG, NOUT], bf16)

            # DRAM loads land directly in ft_t; consumers wait on the DMA
            # queue semaphore (bacc hoists extra matmul waits onto the
            # LDWEIGHTS slot, and the scheduler inserts standalone waits
            # where an instruction needs more than one).
            for lo, hi in in_batches:
                nc.sync.dma_start(ft_t[:, lo:hi], ft_d[:, lo:hi])

            gi = 0
            for t in range(1, K + 1):
                for h in range(2):
                    ps = psum.tile([TAG, CH], f32)
                    o = h * CH
                    rhs = (
                        ft_t[:, bcol(0) + o : bcol(0) + o + CH]
                        if t == 1
                        else snap[:, (t - 2) * CW + o : (t - 2) * CW + o + CH]
                    )
                    nc.tensor.matmul(ps[:], ft_t[:, 0:TAG], rhs)
                    nc.vector.tensor_mul(
                        snap[:, (t - 1) * CW + o : (t - 1) * CW + o + CH],
                        ps[:],
                        ft_t[:, bcol(t) + o : bcol(t) + o + CH],
                    )
                    if t == K:
                        # last block: store each half as soon as its TT lands
                        nc.gpsimd.dma_start(
                            out_d[:, (t - 1) * CW + o : (t - 1) * CW + o + CH],
                            snap[:, (t - 1) * CW + o : (t - 1) * CW + o + CH],
                        )
                while gi < len(out_groups) and out_groups[gi][1] - 1 == t and t < K:
                    # mid-scan groups go on the (idle) input queue so their
                    # descriptor generation never delays the final block's
                    # store on the gpsimd queue
                    a, b2 = out_groups[gi]
                    nc.sync.dma_start(
                        out_d[:, (a - 1) * CW : (b2 - 1) * CW],
                        snap[:, (a - 1) * CW : (b2 - 1) * CW],
                    )
                    gi += 1

    nc.compile()
    return nc


def _get_nc(K):
    if K not in _compiled:
        _compiled[K] = _build_nc(K)
    return _compiled[K]


def _run_device(in_maps, K, trace=False):
    from concourse.bass_utils import run_bass_kernel_spmd

    nc = _get_nc(K)
    return run_bass_kernel_spmd(nc, in_maps, list(range(NCORES)), trace=trace)


def _logsumexp(x, axis=-1):
    m = np.max(x, axis=axis, keepdims=True)
    return np.squeeze(m, axis) + np.log(np.sum(np.exp(x - m), axis=axis))


def prepare_inputs(feats, transitions, s_eff):
    """Host-side prep: normalized emissions packed in stripe order.

    Column layout within a block: col = r*BLOC + bl  (stripe-major).
    Stripe r's chain step k (1..K) applies the emission at absolute time
    t_abs = t0_r + k, clamped to s_eff-1, where t0_0 = 0 and
    t0_r = r*L - W.  Block 0 holds the init states.
    Returns (in_maps, lognorm, p0) — p0 in float64 for the host gather.
    """
    import ml_dtypes

    L, K = _plan(s_eff)
    CW = R * BLOC
    feats64 = feats.astype(np.float64)
    lognorm = _logsumexp(feats64, axis=2)  # (B,S)
    fnorm = np.exp(feats64 - lognorm[:, :, None])  # (B,S,T) float64
    tr = transitions.astype(np.float64)
    e_mat = np.ascontiguousarray(np.exp(tr).astype(np.float32))  # (T,T) rows=i
    es = np.exp(tr[START, :])  # (T,)
    p0 = fnorm[:, 0, :] * es[None, :]  # (B,T) exact init, float64

    t0s = np.array([0] + [r * L - W for r in range(1, R)])  # (R,)
    t_abs = np.clip(t0s[:, None] + np.arange(1, K + 1)[None, :], 0, s_eff - 1)

    bf = ml_dtypes.bfloat16
    in_maps = []
    for c in range(NCORES):
        sl = slice(c * BLOC, (c + 1) * BLOC)
        ftc = np.empty((TAG, TAG + (K + 1) * CW), dtype=bf)
        ftc[:, :TAG] = e_mat.astype(bf)
        blk0 = np.ones((R, BLOC, TAG), dtype=np.float64)
        blk0[0] = p0[sl]
        ftc[:, TAG : TAG + CW] = blk0.reshape(CW, TAG).T.astype(bf)
        sched = fnorm[sl][:, t_abs, :]        # (BLOC, R, K, TAG)
        sched = sched.transpose(3, 2, 1, 0)   # (TAG, K, R, BLOC)
        ftc[:, TAG + CW :] = sched.reshape(TAG, K * CW).astype(bf)
        in_maps.append({"ft": np.ascontiguousarray(ftc)})
    return in_maps, lognorm, p0


def finish(results, lognorm, p0, s_eff, feats, mask, tags, transitions):
    """Calibrate stripe scales, gather per-length states, compute NLL.

    Device out column for the state after chain step k (1..K) of
    (stripe r, lane bl): (k-1)*CW + r*BLOC + bl.
    """
    mask = np.asarray(mask).astype(bool)
    tags = np.asarray(tags).astype(np.int64)
    tr = np.asarray(transitions).astype(np.float64)
    lengths = mask.sum(axis=1).astype(np.int64)
    L, K = _plan(s_eff)
    CW = R * BLOC
    t0s = [0] + [r * L - W for r in range(1, R)]

    fwd = 0.0
    with np.errstate(divide="ignore"):
        for c in range(NCORES):
            out = np.asarray(results[c]["out"]).astype(np.float64)
            for bl in range(BLOC):
                b = c * BLOC + bl
                logscale = np.zeros(R)
                for r in range(1, R):
                    k_r = W                      # stripe r at time r*L
                    k_rm = K if r > 1 else L     # stripe r-1 at time r*L
                    num = out[:, (k_rm - 1) * CW + (r - 1) * BLOC + bl].sum()
                    den = out[:, (k_r - 1) * CW + r * BLOC + bl].sum()
                    logscale[r] = logscale[r - 1] + np.log(num) - np.log(den)
                tb = int(lengths[b]) - 1
                if tb == 0:
                    part = np.log(p0[b]) + lognorm[b, 0]
                else:
                    r = 0 if tb < K else min(tb // L, R - 1)
                    k = tb - t0s[r]              # chain step (1..K)
                    pv = out[:, (k - 1) * CW + r * BLOC + bl]
                    part = np.log(pv) + logscale[r] + lognorm[b, : tb + 1].sum()
                fwd += _logsumexp(part + tr[:, END])

    feats64 = np.asarray(feats).astype(np.float64)
    prev = np.concatenate(
        [np.full((B, 1), START, dtype=np.int64), tags[:, :-1]], axis=1
    )
    emit = np.take_along_axis(feats64, tags[:, :, None], axis=2)[:, :, 0]
    trans_sc = tr[prev, tags]
    tg = np.where(mask, emit + trans_sc, 0.0).sum()
    end_ids = tags[np.arange(B), lengths - 1]
    gold = tg + tr[end_ids, END].sum()

    return np.float32(fwd - gold)


def kernel(feats, mask, tags, transitions):
    feats = np.asarray(feats, dtype=np.float32)
    transitions = np.asarray(transitions, dtype=np.float32)
    s_eff = int(np.asarray(mask).astype(bool).sum(axis=1).max())
    _, K = _plan(s_eff)
    in_maps, lognorm, p0 = prepare_inputs(feats, transitions, s_eff)
    res = _run_device(in_maps, K).results
    return finish(res, lognorm, p0, s_eff, feats, mask, tags, transitions)


# revision 17
# speedup vs baseline: 1.0535x; 1.0027x over previous
"""CRF NLL kernel for Trainium2 (8 NeuronCores, batch-parallel).

Math: the CRF forward recursion
    part_t[j] = logsumexp_i(part_{t-1}[i] + trans[i,j]) + feat[t,j]
is run in the exponential domain:
    p_t[j,b] = (sum_i p_{t-1}[i,b] * E[i,j]) * F_t[j,b]
with E = exp(trans) and F_t = exp(feat_t - lognorm_t) the *normalized*
emission weights (per-(t,b) log-normalizers are folded back in on the
host).

The serial scan over seq_len is broken with a Perron-Frobenius stripe
decomposition: products of strictly positive matrices contract the
projective (Hilbert) metric geometrically — for E = exp(0.1*randn) a
single step washes out the initial direction to below bf16 rounding
noise. Each sequence is split into R overlapping time-stripes; stripe
r>=1 starts from a uniform state W steps before its real region, so
its trajectory equals the true one up to one unknown per-stripe
scalar. The host recovers the scalars by chaining L1-norm ratios at
the overlap times (within-stripe ratios are exact: the scalar
cancels); stripe 0 starts from the exact p_0, anchoring the absolute
scale.

Device work: K = L+W steps of one [64x64] @ [64 x 8*R] bf16 matmul
(E stationary) plus one DVE multiply PSUM*F -> bf16 SBUF, instead of
s_eff serial steps of an 8-wide matmul. Chain length 9 instead of 252;
per-step instruction overheads (PE fixed SBUF access ~173ns, DVE PSUM
access ~125ns, semaphores) dominate, so the 32x wider ops are nearly
free. The host pre-packs the per-(stripe, seq) emission schedule so
the device kernel is a plain dense scan. Inputs arrive in two batched
DMAs; trajectory blocks are stored in three grouped DMAs streamed
behind the scan.
"""

import sys

sys.path.insert(0, "/opt/trn_rl_repo")

import numpy as np

B, S, TAG = 64, 256, 64
START, END = TAG - 2, TAG - 1
NCORES = 8
BLOC = B // NCORES  # 8 sequences per core

R = 96  # stripes per sequence
W = 1   # warmup steps per stripe

_compiled = {}


def _plan(s_eff):
    """Stripe geometry: L real steps per stripe, K=L+W chain steps."""
    L = max(1, -(-(s_eff - W) // R))  # ceil((s_eff-W)/R)
    K = L + W
    return L, K


def _build_nc(K):
    import concourse.bass as bass
    import concourse.bacc as bacc
    import concourse.mybir as mybir
    from concourse import tile

    f32 = mybir.dt.float32
    bf16 = mybir.dt.bfloat16
    nc = bacc.Bacc(
        "TRN2", target_bir_lowering=False, debug=False, num_devices=NCORES
    )

    CW = R * BLOC                   # columns per step-block
    NIN = TAG + (K + 1) * CW        # [E | init block | step blocks 1..K]
    NOUT = K * CW                   # states after steps 1..K
    ft_d = nc.dram_tensor("ft", [TAG, NIN], bf16, kind="ExternalInput")
    out_d = nc.dram_tensor("out", [TAG, NOUT], bf16, kind="ExternalOutput")

    def bcol(k):  # first ft column of step-block k
        return TAG + k * CW

    # input DMA batches (column ranges): E + the first two blocks arrive in
    # one transfer so a single DMA completion gates the first matmul; the
    # rest is split so later blocks' semaphores land before their TT needs
    # them (per-DMA latency is ~2.2us: DGE gen + engine delay + sem prop).
    in_batches = [(0, bcol(1)), (bcol(1), bcol(2))] + [
        (bcol(k), bcol(min(k + 2, K + 1))) for k in range(2, K + 1, 2)
    ]
    in_batches = [(a, b) for a, b in in_batches if a < b]
    # output DMA groups (1-indexed step blocks), issued as the scan passes;
    # the last group is a single block (stored per half-chain) to shorten
    # the post-scan tail
    out_groups = [(1, K // 2), (K // 2, K), (K, K + 1)]
    out_groups = [(a, b) for a, b in out_groups if a < b]

    CH = CW // 2  # per-chain width: two interleaved chains overlap PE and DVE

    with tile.TileContext(nc) as tc:
        with (
            tc.tile_pool(name="pool", bufs=1) as pool,
            tc.tile_pool(name="psum", bufs=4, space=bass.MemorySpace.PSUM) as psum,
        ):
            ft_t = pool.tile([TAG, NIN], bf16)
            snap = pool.tile([TAG, NOUT], bf16)

            # DRAM loads land directly in ft_t; consumers wait on the DMA
            # queue semaphore (bacc hoists extra matmul waits onto the
            # LDWEIGHTS slot, and the scheduler inserts standalone waits
            # where an instruction needs more than one).
            for lo, hi in in_batches:
                nc.sync.dma_start(ft_t[:, lo:hi], ft_d[:, lo:hi])

            gi = 0
            for t in range(1, K + 1):
                for h in range(2):
                    ps = psum.tile([TAG, CH], f32)
                    o = h * CH
                    rhs = (
                        ft_t[:, bcol(0) + o : bcol(0) + o + CH]
                        if t == 1
                        else snap[:, (t - 2) * CW + o : (t - 2) * CW + o + CH]
                    )
                    nc.tensor.matmul(ps[:], ft_t[:, 0:TAG], rhs)
                    nc.vector.tensor_mul(
                        snap[:, (t - 1) * CW + o : (t - 1) * CW + o + CH],
                        ps[:],
                        ft_t[:, bcol(t) + o : bcol(t) + o + CH],
                    )
                if t == K and h == 1:
                    nc.gpsimd.dma_start(
                        out_d[:, (t - 1) * CW : t * CW],
                        snap[:, (t - 1) * CW : t * CW],
                    )
                while gi < len(out_groups) and out_groups[gi][1] - 1 == t and t < K:
                    # mid-scan groups go on the (idle) input queue so their
                    # descriptor generation never delays the final block's
                    # store on the gpsimd queue
                    a, b2 = out_groups[gi]
                    nc.sync.dma_start(
                        out_d[:, (a - 1) * CW : (b2 - 1) * CW],
                        snap[:, (a - 1) * CW : (b2 - 1) * CW],
                    )
                    gi += 1

    nc.compile()
    return nc


def _get_nc(K):
    if K not in _compiled:
        _compiled[K] = _build_nc(K)
    return _compiled[K]


def _run_device(in_maps, K, trace=False):
    from concourse.bass_utils import run_bass_kernel_spmd

    nc = _get_nc(K)
    return run_bass_kernel_spmd(nc, in_maps, list(range(NCORES)), trace=trace)


def _logsumexp(x, axis=-1):
    m = np.max(x, axis=axis, keepdims=True)
    return np.squeeze(m, axis) + np.log(np.sum(np.exp(x - m), axis=axis))


def prepare_inputs(feats, transitions, s_eff):
    """Host-side prep: normalized emissions packed in stripe order.

    Column layout within a block: col = r*BLOC + bl  (stripe-major).
    Stripe r's chain step k (1..K) applies the emission at absolute time
    t_abs = t0_r + k, clamped to s_eff-1, where t0_0 = 0 and
    t0_r = r*L - W.  Block 0 holds the init states.
    Returns (in_maps, lognorm, p0) — p0 in float64 for the host gather.
    """
    import ml_dtypes

    L, K = _plan(s_eff)
    CW = R * BLOC
    feats64 = feats.astype(np.float64)
    lognorm = _logsumexp(feats64, axis=2)  # (B,S)
    fnorm = np.exp(feats64 - lognorm[:, :, None])  # (B,S,T) float64
    tr = transitions.astype(np.float64)
    e_mat = np.ascontiguousarray(np.exp(tr).astype(np.float32))  # (T,T) rows=i
    es = np.exp(tr[START, :])  # (T,)
    p0 = fnorm[:, 0, :] * es[None, :]  # (B,T) exact init, float64

    t0s = np.array([0] + [r * L - W for r in range(1, R)])  # (R,)
    t_abs = np.clip(t0s[:, None] + np.arange(1, K + 1)[None, :], 0, s_eff - 1)

    bf = ml_dtypes.bfloat16
    in_maps = []
    for c in range(NCORES):
        sl = slice(c * BLOC, (c + 1) * BLOC)
        ftc = np.empty((TAG, TAG + (K + 1) * CW), dtype=bf)
        ftc[:, :TAG] = e_mat.astype(bf)
        blk0 = np.ones((R, BLOC, TAG), dtype=np.float64)
        blk0[0] = p0[sl]
        ftc[:, TAG : TAG + CW] = blk0.reshape(CW, TAG).T.astype(bf)
        sched = fnorm[sl][:, t_abs, :]        # (BLOC, R, K, TAG)
        sched = sched.transpose(3, 2, 1, 0)   # (TAG, K, R, BLOC)
        ftc[:, TAG + CW :] = sched.reshape(TAG, K * CW).astype(bf)
        in_maps.append({"ft": np.ascontiguousarray(ftc)})
    return in_maps, lognorm, p0


def finish(results, lognorm, p0, s_eff, feats, mask, tags, transitions):
    """Calibrate stripe scales, gather per-length states, compute NLL.

    Device out column for the state after chain step k (1..K) of
    (stripe r, lane bl): (k-1)*CW + r*BLOC + bl.
    """
    mask = np.asarray(mask).astype(bool)
    tags = np.asarray(tags).astype(np.int64)
    tr = np.asarray(transitions).astype(np.float64)
    lengths = mask.sum(axis=1).astype(np.int64)
    L, K = _plan(s_eff)
    CW = R * BLOC
    t0s = [0] + [r * L - W for r in range(1, R)]

    fwd = 0.0
    with np.errstate(divide="ignore"):
        for c in range(NCORES):
            out = np.asarray(results[c]["out"]).astype(np.float64)
            for bl in range(BLOC):
                b = c * BLOC + bl
                logscale = np.zeros(R)
                for r in range(1, R):
                    k_r = W                      # stripe r at time r*L
                    k_rm = K if r > 1 else L     # stripe r-1 at time r*L
                    num = out[:, (k_rm - 1) * CW + (r - 1) * BLOC + bl].sum()
                    den = out[:, (k_r - 1) * CW + r * BLOC + bl].sum()
                    logscale[r] = logscale[r - 1] + np.log(num) - np.log(den)
                tb = int(lengths[b]) - 1
                if tb == 0:
                    part = np.log(p0[b]) + lognorm[b, 0]
                else:
                    r = 0 if tb < K else min(tb // L, R - 1)
                    k = tb - t0s[r]              # chain step (1..K)
                    pv = out[:, (k - 1) * CW + r * BLOC + bl]
                    part = np.log(pv) + logscale[r] + lognorm[b, : tb + 1].sum()
                fwd += _logsumexp(part + tr[:, END])

    feats64 = np.asarray(feats).astype(np.float64)
    prev = np.concatenate(
        [np.full((B, 1), START, dtype=np.int64), tags[:, :-1]], axis=1
    )
    emit = np.take_along_axis(feats64, tags[:, :, None], axis=2)[:, :, 0]
    trans_sc = tr[prev, tags]
    tg = np.where(mask, emit + trans_sc, 0.0).sum()
    end_ids = tags[np.arange(B), lengths - 1]
    gold = tg + tr[end_ids, END].sum()

    return np.float32(fwd - gold)


def kernel(feats, mask, tags, transitions):
    feats = np.asarray(feats, dtype=np.float32)
    transitions = np.asarray(transitions, dtype=np.float32)
    s_eff = int(np.asarray(mask).astype(bool).sum(axis=1).max())
    _, K = _plan(s_eff)
    in_maps, lognorm, p0 = prepare_inputs(feats, transitions, s_eff)
    res = _run_device(in_maps, K).results
    return finish(res, lognorm, p0, s_eff, feats, mask, tags, transitions)


# revision 18
# speedup vs baseline: 1.0659x; 1.0118x over previous
"""CRF NLL kernel for Trainium2 (8 NeuronCores, batch-parallel).

Math: the CRF forward recursion
    part_t[j] = logsumexp_i(part_{t-1}[i] + trans[i,j]) + feat[t,j]
is run in the exponential domain:
    p_t[j,b] = (sum_i p_{t-1}[i,b] * E[i,j]) * F_t[j,b]
with E = exp(trans) and F_t = exp(feat_t - lognorm_t) the *normalized*
emission weights (per-(t,b) log-normalizers are folded back in on the
host).

The serial scan over seq_len is broken with a Perron-Frobenius stripe
decomposition: products of strictly positive matrices contract the
projective (Hilbert) metric geometrically — for E = exp(0.1*randn) a
single step washes out the initial direction to below bf16 rounding
noise. Each sequence is split into R overlapping time-stripes; stripe
r>=1 starts from a uniform state W steps before its real region, so
its trajectory equals the true one up to one unknown per-stripe
scalar. The host recovers the scalars by chaining L1-norm ratios at
the overlap times (within-stripe ratios are exact: the scalar
cancels); stripe 0 starts from the exact p_0, anchoring the absolute
scale.

Device work: K = L+W steps of one [64x64] @ [64 x 8*R] bf16 matmul
(E stationary) plus one DVE multiply PSUM*F -> bf16 SBUF, instead of
s_eff serial steps of an 8-wide matmul. Chain length 9 instead of 252;
per-step instruction overheads (PE fixed SBUF access ~173ns, DVE PSUM
access ~125ns, semaphores) dominate, so the 32x wider ops are nearly
free. The host pre-packs the per-(stripe, seq) emission schedule so
the device kernel is a plain dense scan. Inputs arrive in two batched
DMAs; trajectory blocks are stored in three grouped DMAs streamed
behind the scan.
"""

import sys

sys.path.insert(0, "/opt/trn_rl_repo")

import numpy as np

B, S, TAG = 64, 256, 64
START, END = TAG - 2, TAG - 1
NCORES = 8
BLOC = B // NCORES  # 8 sequences per core

R = 64  # stripes per sequence
W = 1   # warmup steps per stripe

_compiled = {}


def _plan(s_eff):
    """Stripe geometry: L real steps per stripe, K=L+W chain steps."""
    L = max(1, -(-(s_eff - W) // R))  # ceil((s_eff-W)/R)
    K = L + W
    return L, K


def _build_nc(K):
    import concourse.bass as bass
    import concourse.bacc as bacc
    import concourse.mybir as mybir
    from concourse import tile

    f32 = mybir.dt.float32
    bf16 = mybir.dt.bfloat16
    nc = bacc.Bacc(
        "TRN2", target_bir_lowering=False, debug=False, num_devices=NCORES
    )

    CW = R * BLOC                   # columns per step-block
    NIN = TAG + (K + 1) * CW        # [E | init block | step blocks 1..K]
    NOUT = K * CW                   # states after steps 1..K
    ft_d = nc.dram_tensor("ft", [TAG, NIN], bf16, kind="ExternalInput")
    out_d = nc.dram_tensor("out", [TAG, NOUT], bf16, kind="ExternalOutput")

    def bcol(k):  # first ft column of step-block k
        return TAG + k * CW

    # input DMA batches (column ranges): E + the first two blocks arrive in
    # one transfer so a single DMA completion gates the first matmul; the
    # rest is split so later blocks' semaphores land before their TT needs
    # them (per-DMA latency is ~2.2us: DGE gen + engine delay + sem prop).
    in_batches = [(0, bcol(1)), (bcol(1), bcol(2))] + [
        (bcol(k), bcol(min(k + 2, K + 1))) for k in range(2, K + 1, 2)
    ]
    in_batches = [(a, b) for a, b in in_batches if a < b]
    # output DMA groups (1-indexed step blocks), issued as the scan passes;
    # the last group is a single block (stored per half-chain) to shorten
    # the post-scan tail
    out_groups = [(1, K // 2), (K // 2, K), (K, K + 1)]
    out_groups = [(a, b) for a, b in out_groups if a < b]

    CH = CW // 2  # per-chain width: two interleaved chains overlap PE and DVE

    with tile.TileContext(nc) as tc:
        with (
            tc.tile_pool(name="pool", bufs=1) as pool,
            tc.tile_pool(name="psum", bufs=4, space=bass.MemorySpace.PSUM) as psum,
        ):
            ft_t = pool.tile([TAG, NIN], bf16)
            snap = pool.tile([TAG, NOUT], bf16)

            # DRAM loads land directly in ft_t; consumers wait on the DMA
            # queue semaphore (bacc hoists extra matmul waits onto the
            # LDWEIGHTS slot, and the scheduler inserts standalone waits
            # where an instruction needs more than one).
            for lo, hi in in_batches:
                nc.sync.dma_start(ft_t[:, lo:hi], ft_d[:, lo:hi])

            gi = 0
            for t in range(1, K + 1):
                for h in range(2):
                    ps = psum.tile([TAG, CH], f32)
                    o = h * CH
                    rhs = (
                        ft_t[:, bcol(0) + o : bcol(0) + o + CH]
                        if t == 1
                        else snap[:, (t - 2) * CW + o : (t - 2) * CW + o + CH]
                    )
                    nc.tensor.matmul(ps[:], ft_t[:, 0:TAG], rhs)
                    nc.vector.tensor_mul(
                        snap[:, (t - 1) * CW + o : (t - 1) * CW + o + CH],
                        ps[:],
                        ft_t[:, bcol(t) + o : bcol(t) + o + CH],
                    )
                if t == K and h == 1:
                    nc.gpsimd.dma_start(
                        out_d[:, (t - 1) * CW : t * CW],
                        snap[:, (t - 1) * CW : t * CW],
                    )
                while gi < len(out_groups) and out_groups[gi][1] - 1 == t and t < K:
                    # mid-scan groups go on the (idle) input queue so their
                    # descriptor generation never delays the final block's
                    # store on the gpsimd queue
                    a, b2 = out_groups[gi]
                    nc.sync.dma_start(
                        out_d[:, (a - 1) * CW : (b2 - 1) * CW],
                        snap[:, (a - 1) * CW : (b2 - 1) * CW],
                    )
                    gi += 1

    nc.compile()
    return nc


def _get_nc(K):
    if K not in _compiled:
        _compiled[K] = _build_nc(K)
    return _compiled[K]


def _run_device(in_maps, K, trace=False):
    from concourse.bass_utils import run_bass_kernel_spmd

    nc = _get_nc(K)
    return run_bass_kernel_spmd(nc, in_maps, list(range(NCORES)), trace=trace)


def _logsumexp(x, axis=-1):
    m = np.max(x, axis=axis, keepdims=True)
    return np.squeeze(m, axis) + np.log(np.sum(np.exp(x - m), axis=axis))


def prepare_inputs(feats, transitions, s_eff):
    """Host-side prep: normalized emissions packed in stripe order.

    Column layout within a block: col = r*BLOC + bl  (stripe-major).
    Stripe r's chain step k (1..K) applies the emission at absolute time
    t_abs = t0_r + k, clamped to s_eff-1, where t0_0 = 0 and
    t0_r = r*L - W.  Block 0 holds the init states.
    Returns (in_maps, lognorm, p0) — p0 in float64 for the host gather.
    """
    import ml_dtypes

    L, K = _plan(s_eff)
    CW = R * BLOC
    feats64 = feats.astype(np.float64)
    lognorm = _logsumexp(feats64, axis=2)  # (B,S)
    fnorm = np.exp(feats64 - lognorm[:, :, None])  # (B,S,T) float64
    tr = transitions.astype(np.float64)
    e_mat = np.ascontiguousarray(np.exp(tr).astype(np.float32))  # (T,T) rows=i
    es = np.exp(tr[START, :])  # (T,)
    p0 = fnorm[:, 0, :] * es[None, :]  # (B,T) exact init, float64

    t0s = np.array([0] + [r * L - W for r in range(1, R)])  # (R,)
    t_abs = np.clip(t0s[:, None] + np.arange(1, K + 1)[None, :], 0, s_eff - 1)

    bf = ml_dtypes.bfloat16
    in_maps = []
    for c in range(NCORES):
        sl = slice(c * BLOC, (c + 1) * BLOC)
        ftc = np.empty((TAG, TAG + (K + 1) * CW), dtype=bf)
        ftc[:, :TAG] = e_mat.astype(bf)
        blk0 = np.ones((R, BLOC, TAG), dtype=np.float64)
        blk0[0] = p0[sl]
        ftc[:, TAG : TAG + CW] = blk0.reshape(CW, TAG).T.astype(bf)
        sched = fnorm[sl][:, t_abs, :]        # (BLOC, R, K, TAG)
        sched = sched.transpose(3, 2, 1, 0)   # (TAG, K, R, BLOC)
        ftc[:, TAG + CW :] = sched.reshape(TAG, K * CW).astype(bf)
        in_maps.append({"ft": np.ascontiguousarray(ftc)})
    return in_maps, lognorm, p0


def finish(results, lognorm, p0, s_eff, feats, mask, tags, transitions):
    """Calibrate stripe scales, gather per-length states, compute NLL.

    Device out column for the state after chain step k (1..K) of
    (stripe r, lane bl): (k-1)*CW + r*BLOC + bl.
    """
    mask = np.asarray(mask).astype(bool)
    tags = np.asarray(tags).astype(np.int64)
    tr = np.asarray(transitions).astype(np.float64)
    lengths = mask.sum(axis=1).astype(np.int64)
    L, K = _plan(s_eff)
    CW = R * BLOC
    t0s = [0] + [r * L - W for r in range(1, R)]

    fwd = 0.0
    with np.errstate(divide="ignore"):
        for c in range(NCORES):
            out = np.asarray(results[c]["out"]).astype(np.float64)
            for bl in range(BLOC):
                b = c * BLOC + bl
                logscale = np.zeros(R)
                for r in range(1, R):
                    k_r = W                      # stripe r at time r*L
                    k_rm = K if r > 1 else L     # stripe r-1 at time r*L
                    num = out[:, (k_rm - 1) * CW + (r - 1) * BLOC + bl].sum()
                    den = out[:, (k_r - 1) * CW + r * BLOC + bl].sum()
                    logscale[r] = logscale[r - 1] + np.log(num) - np.log(den)
                tb = int(lengths[b]) - 1
                if tb == 0:
                    part = np.log(p0[b]) + lognorm[b, 0]
                else:
                    r = 0 if tb < K else min(tb // L, R - 1)
                    k = tb - t0s[r]              # chain step (1..K)
                    pv = out[:, (k - 1) * CW + r * BLOC + bl]
                    part = np.log(pv) + logscale[r] + lognorm[b, : tb + 1].sum()
                fwd += _logsumexp(part + tr[:, END])

    feats64 = np.asarray(feats).astype(np.float64)
    prev = np.concatenate(
        [np.full((B, 1), START, dtype=np.int64), tags[:, :-1]], axis=1
    )
    emit = np.take_along_axis(feats64, tags[:, :, None], axis=2)[:, :, 0]
    trans_sc = tr[prev, tags]
    tg = np.where(mask, emit + trans_sc, 0.0).sum()
    end_ids = tags[np.arange(B), lengths - 1]
    gold = tg + tr[end_ids, END].sum()

    return np.float32(fwd - gold)


def kernel(feats, mask, tags, transitions):
    feats = np.asarray(feats, dtype=np.float32)
    transitions = np.asarray(transitions, dtype=np.float32)
    s_eff = int(np.asarray(mask).astype(bool).sum(axis=1).max())
    _, K = _plan(s_eff)
    in_maps, lognorm, p0 = prepare_inputs(feats, transitions, s_eff)
    res = _run_device(in_maps, K).results
    return finish(res, lognorm, p0, s_eff, feats, mask, tags, transitions)
